# revision 4
# baseline (speedup 1.0000x reference)
"""Trainium2 Bass kernel for nn_CLIP_Inplanted_groupPNmixAfterConv_groupMaxNensembleOut.

Math (derived from the reference):
  For level l (g = 2**l groups, gc = 1024/g channels each),
  mix_l = a*x + b*xp + c per (b, group, s) with
    a = 0.5*s2/s1, b = 0.5*s1/s2, c = 0.5*(m1+m2) - a*m1 - b*m2.
  Identities: a*b = 1/4; sum(mix_l) is level-independent; and
    sumsq(mix_l) = (Q1t+Q2t)/4 + Pt/2 - (EPS/4)*D_l,
    D_l = sum (gc-1)*(rho + 1/rho - 2),  rho = (V2+EPS)/(V1+EPS).
  So topk-by-variance == bottomk-by-D, robustly computable in fp32, and
  out = A*x + B*xp + C with A,B,C the mean over selected levels of (a,b,c)
  broadcast to the finest 8-channel grid.

Device plan (8 cores, batch-sharded rows; perm partners gathered on host):
  slab layout [part = channel-of-slab (128), free = spatial (289)].
  NEFF1 (stats): PE fp32r indicator matmuls -> per-level S,Q; ACT exp/log
    pipeline -> a',b',c' fields (HBM spill) + per-level D partials + global
    partials.  Host: fp64 scores, stable top-3, masks.
  NEFF2 (apply): masked select matmuls collapse fields, per-slab indicator
    matmuls upsample, out = A*xa + B*xb + C on DVE/gpsimd.
Inputs are host-rounded to the fp32r (tf32) grid; PE fp32r is then exact.
"""

import numpy as np

B, C, H, W = 64, 1024, 17, 17
S = H * W            # 289
S2 = S + 1           # padded even spatial extent for fp32r matmuls
R = 8                # batch rows per core
NCORES = 8
NSLAB = 8
EPS = 1e-5
NF = R * S2          # 2320 free elems in row-batched level tiles
_cache = {}


def _round_fp32r(a, bits=13):
    ai = a.view(np.uint32).astype(np.uint64)
    half = np.uint64(1 << (bits - 1))
    mask = np.uint64(~((1 << bits) - 1) & 0xFFFFFFFF)
    return ((ai + half) & mask).astype(np.uint32).view(np.float32)


def _consts():
    ind7 = np.zeros((NSLAB, 128, 128), dtype=np.float32)
    for j in range(NSLAB):
        for c in range(128):
            ind7[j, c, 16 * j + c // 8] = 1.0
    eup = np.ascontiguousarray(ind7.transpose(0, 2, 1))
    ind127 = np.zeros((128, 127), dtype=np.float32)
    nvec = np.zeros(127, dtype=np.float64)
    for lvl in range(7):
        off = 2 ** lvl - 1
        glen = 128 >> lvl
        for i in range(2 ** lvl):
            ind127[i * glen:(i + 1) * glen, off + i] = 1.0
            nvec[off + i] = 1024 >> lvl
    up07 = np.ascontiguousarray(ind127.T)
    return ind7, eup, ind127, up07, nvec


def _build_neff1():
    import concourse.bacc as bacc
    import concourse.mybir as mybir
    import concourse.tile as tile

    F32 = mybir.dt.float32
    F32R = mybir.dt.float32r
    OP = mybir.AluOpType
    ACTF = mybir.ActivationFunctionType
    AX = mybir.AxisListType

    nc = bacc.Bacc("TRN2", target_bir_lowering=False, num_devices=NCORES)

    xa_d = nc.dram_tensor("xa", [R, NSLAB, 128, S2], F32R, kind="ExternalInput")
    xb_d = nc.dram_tensor("xb", [R, NSLAB, 128, S2], F32R, kind="ExternalInput")
    ind7_d = nc.dram_tensor("ind7", [NSLAB, 128, 128], F32R, kind="ExternalInput")
    ind127_d = nc.dram_tensor("ind127", [128, 127], F32R, kind="ExternalInput")
    ident_d = nc.dram_tensor("ident", [128, 128], F32R, kind="ExternalInput")
    nv_names = ["invsq", "invnm1", "lnnm1", "nega", "c6n"]
    nv_d = {k: nc.dram_tensor(f"nv_{k}", [127, 1], F32, kind="ExternalInput")
            for k in nv_names}

    dpart0_d = nc.dram_tensor("dpart0", [127, 1], F32, kind="ExternalOutput")
    dpart1_d = nc.dram_tensor("dpart1", [128, 1], F32, kind="ExternalOutput")
    qa_d = nc.dram_tensor("qa", [128, R], F32, kind="ExternalOutput")
    cr_d = nc.dram_tensor("cr", [128, R], F32, kind="ExternalOutput")
    ssum_d = nc.dram_tensor("ssum", [127, 1], F32, kind="ExternalOutput")
    fshape = [[127, NF], [128, NF]]
    af_d = [nc.dram_tensor(f"af{g}", fshape[g], F32, kind="ExternalOutput")
            for g in range(2)]
    bf_d = [nc.dram_tensor(f"bf{g}", fshape[g], F32, kind="ExternalOutput")
            for g in range(2)]
    cf_d = [nc.dram_tensor(f"cf{g}", fshape[g], F32, kind="ExternalOutput")
            for g in range(2)]

    with tile.TileContext(nc) as tc:
        with (
            tc.tile_pool(name="consts", bufs=1) as cpool,
            tc.tile_pool(name="rows", bufs=2) as rpool,
            tc.tile_pool(name="scr", bufs=1) as scrpool,
            tc.tile_pool(name="stats", bufs=1) as spool,
            tc.tile_pool(name="pipe", bufs=1) as ppool,
            tc.tile_pool(name="ps_l", bufs=1, space="PSUM") as ps1,
            tc.tile_pool(name="ps_u", bufs=1, space="PSUM") as ps2,
        ):
            ind7_t = cpool.tile([128, NSLAB, 128], F32R, name="ind7_t")
            nc.sync.dma_start(ind7_t[:], ind7_d[:, :, :].rearrange("j c k -> c j k"))
            ind127_t = cpool.tile([128, 127], F32R, name="ind127_t")
            nc.sync.dma_start(ind127_t[:], ind127_d[:, :])
            ident_t = cpool.tile([128, 128], F32R, name="ident_t")
            nc.sync.dma_start(ident_t[:], ident_d[:, :])
            nv_t = {}
            for k in nv_names:
                nv_t[k] = cpool.tile([127, 1], F32, name=f"nv_{k}_t")
                nc.sync.dma_start(nv_t[k][:], nv_d[k][:, :])
            eps_t = cpool.tile([128, 1], F32, name="eps_t")
            nc.vector.memset(eps_t[:], EPS)
            ln6_t = cpool.tile([128, 1], F32, name="ln6_t")
            nc.vector.memset(ln6_t[:], float(-np.log(6.0)))
            ln7_t = cpool.tile([128, 1], F32, name="ln7_t")
            nc.vector.memset(ln7_t[:], float(np.log(7.0)))

            LS = {}
            for st in ["s1", "q1", "s2", "q2"]:
                LS[(st, 0)] = spool.tile([127, NF], F32, name=f"L0_{st}")
                LS[(st, 1)] = spool.tile([128, NF], F32, name=f"L1_{st}")

            # ---------------- per-row stats ----------------
            for r in range(R):
                xa_t = rpool.tile([128, NSLAB, S2], F32R, name="xa_t")
                nc.sync.dma_start(xa_t[:], xa_d[r].rearrange("j c s -> c j s"))
                xb_t = rpool.tile([128, NSLAB, S2], F32R, name="xb_t")
                nc.sync.dma_start(xb_t[:], xb_d[r].rearrange("j c s -> c j s"))

                # cross term: u = xa + xb accumulated in PSUM via identity MMs
                psu = ps2.tile([128, NF], F32, name="psu")
                xaf = xa_t[:].rearrange("c j s -> c (j s)")
                xbf = xb_t[:].rearrange("c j s -> c (j s)")
                for ch in range(5):
                    lo = ch * 512
                    hi = min(NF, lo + 512)
                    nc.tensor.matmul(psu[:, lo:hi], ident_t[:], xaf[:, lo:hi],
                                     start=True, stop=False)
                    nc.tensor.matmul(psu[:, lo:hi], ident_t[:], xbf[:, lo:hi],
                                     start=False, stop=True)
                cr_t = rpool.tile([128, 1], F32, name="cr_t")
                usq_t = scrpool.tile([128, NF], F32, name="usq_t")
                nc.scalar.activation(usq_t[:], psu[:], ACTF.Square,
                                     accum_out=cr_t[:])
                nc.sync.dma_start(cr_d[:, r:r + 1], cr_t[:])

                # squares (+ Q1 partial accum on the xa square)
                qa_t = rpool.tile([128, 1], F32, name="qa_t")

                def stats_for(data_t, s_key, q_key, acc):
                    sq_t = rpool.tile([128, NSLAB, S2], F32R, name="sq_t")
                    nc.scalar.activation(sq_t[:], data_t[:].bitcast(F32),
                                         ACTF.Square, accum_out=acc)
                    for st, dat in [(s_key, data_t), (q_key, sq_t)]:
                        ps7 = ps1.tile([128, S2], F32, name="ps7")
                        for j in range(NSLAB):
                            nc.tensor.matmul(ps7[:], ind7_t[:, j, :],
                                             dat[:, j, :],
                                             start=(j == 0), stop=(j == NSLAB - 1))
                        f7dst = LS[(st, 1)][:, r * S2:(r + 1) * S2]
                        nc.vector.tensor_copy(f7dst.bitcast(F32R), ps7[:])
                        ps06 = ps1.tile([127, S2], F32, name="ps06")
                        nc.tensor.matmul(ps06[:], ind127_t[:],
                                         f7dst.bitcast(F32R),
                                         start=True, stop=True)
                        nc.scalar.copy(LS[(st, 0)][:, r * S2:(r + 1) * S2], ps06[:])

                stats_for(xa_t, "s1", "q1", qa_t[:])
                nc.sync.dma_start(qa_d[:, r:r + 1], qa_t[:])
                stats_for(xb_t, "s2", "q2", None)

            # ---------------- level pipeline ----------------
            for g in range(2):
                P = 127 if g == 0 else 128
                s1 = LS[("s1", g)]; q1 = LS[("q1", g)]
                s2 = LS[("s2", g)]; q2 = LS[("q2", g)]
                if g == 0:
                    invsq = nv_t["invsq"][:]
                    invnm1 = nv_t["invnm1"][:]
                    lnnm1 = nv_t["lnnm1"][:]
                    nega = nv_t["nega"][:]
                    c6n = nv_t["c6n"][:]
                else:
                    invsq = float(1.0 / np.sqrt(56.0))
                    invnm1 = float(1.0 / 7.0)
                    lnnm1 = ln7_t[:]
                    nega = float(-1.0 / 8.0)
                    c6n = float(1.0 / 48.0)

                msq = ppool.tile([128, NF], F32, name="msq")
                vA = ppool.tile([128, NF], F32, name="vA")
                vB = ppool.tile([128, NF], F32, name="vB")
                d_t = ppool.tile([128, NF], F32, name="d_t")
                ap_t = ppool.tile([128, NF], F32, name="ap_t")
                bp_t = ppool.tile([128, NF], F32, name="bp_t")

                # V1 -> ln(V1+eps) in vA
                nc.scalar.activation(msq[:P], s1[:], ACTF.Square, scale=invsq)
                nc.vector.scalar_tensor_tensor(
                    out=vA[:P], in0=q1[:], scalar=invnm1, op0=OP.mult,
                    in1=msq[:P], op1=OP.subtract)
                nc.vector.tensor_scalar_max(out=vA[:P], in0=vA[:P], scalar1=0.0)
                nc.scalar.activation(vA[:P], vA[:P], ACTF.Ln, bias=eps_t[:P])
                # V2 -> ln(V2+eps) in vB
                nc.scalar.activation(msq[:P], s2[:], ACTF.Square, scale=invsq)
                nc.vector.scalar_tensor_tensor(
                    out=vB[:P], in0=q2[:], scalar=invnm1, op0=OP.mult,
                    in1=msq[:P], op1=OP.subtract)
                nc.vector.tensor_scalar_max(out=vB[:P], in0=vB[:P], scalar1=0.0)
                nc.scalar.activation(vB[:P], vB[:P], ACTF.Ln, bias=eps_t[:P])

                nc.vector.tensor_tensor(out=d_t[:P], in0=vB[:P], in1=vA[:P],
                                        op=OP.subtract)

                # D partials: (n-1)(e^d + e^-d), -2(n-1) constant on host
                nc.scalar.activation(vA[:P], d_t[:P], ACTF.Exp, scale=1.0,
                                     bias=lnnm1)
                nc.scalar.activation(vB[:P], d_t[:P], ACTF.Exp, scale=-1.0,
                                     bias=lnnm1)
                zacc = ppool.tile([128, 1], F32, name="zacc")
                nc.vector.scalar_tensor_tensor(
                    out=msq[:P], in0=vA[:P], scalar=1.0, op0=OP.bypass,
                    in1=vB[:P], op1=OP.add, accum_out=zacc[:P])
                nc.sync.dma_start((dpart0_d if g == 0 else dpart1_d)[:, :],
                                  zacc[:P])

                # a', b'
                nc.scalar.activation(ap_t[:P], d_t[:P], ACTF.Exp, scale=0.5,
                                     bias=ln6_t[:P])
                nc.scalar.activation(bp_t[:P], d_t[:P], ACTF.Exp, scale=-0.5,
                                     bias=ln6_t[:P])
                nc.sync.dma_start(af_d[g][:, :], ap_t[:P])
                nc.sync.dma_start(bf_d[g][:, :], bp_t[:P])

                # c' = (1/(6n) - a'/n)*S1 + (1/(6n) - b'/n)*S2
                nc.vector.tensor_scalar(out=vA[:P], in0=ap_t[:P],
                                        scalar1=nega, scalar2=c6n,
                                        op0=OP.mult, op1=OP.add)
                nc.vector.tensor_scalar(out=vB[:P], in0=bp_t[:P],
                                        scalar1=nega, scalar2=c6n,
                                        op0=OP.mult, op1=OP.add)
                nc.gpsimd.tensor_tensor(out=msq[:P], in0=vA[:P], in1=s1[:],
                                        op=OP.mult)
                nc.gpsimd.tensor_tensor(out=d_t[:P], in0=vB[:P], in1=s2[:],
                                        op=OP.mult)
                nc.gpsimd.tensor_tensor(out=vA[:P], in0=msq[:P], in1=d_t[:P],
                                        op=OP.add)
                nc.sync.dma_start(cf_d[g][:, :], vA[:P])

                if g == 0:
                    ss_t = ppool.tile([127, 1], F32, name="ss_t")
                    nc.vector.reduce_sum(ss_t[:], s1[:], axis=AX.X)
                    nc.sync.dma_start(ssum_d[:, :], ss_t[:])

    nc.finalize()
    return nc


def _build_neff2():
    import concourse.bacc as bacc
    import concourse.mybir as mybir
    import concourse.tile as tile

    F32 = mybir.dt.float32
    F32R = mybir.dt.float32r
    OP = mybir.AluOpType

    nc = bacc.Bacc("TRN2", target_bir_lowering=False, num_devices=NCORES)

    xa_d = nc.dram_tensor("xa", [R, NSLAB, 128, S2], F32R, kind="ExternalInput")
    xb_d = nc.dram_tensor("xb", [R, NSLAB, 128, S2], F32R, kind="ExternalInput")
    fshape = [[127, NF], [128, NF]]
    af_d = [nc.dram_tensor(f"af{g}", fshape[g], F32R, kind="ExternalInput")
            for g in range(2)]
    bf_d = [nc.dram_tensor(f"bf{g}", fshape[g], F32R, kind="ExternalInput")
            for g in range(2)]
    cf_d = [nc.dram_tensor(f"cf{g}", fshape[g], F32R, kind="ExternalInput")
            for g in range(2)]
    up07_d = nc.dram_tensor("up07", [127, 128], F32R, kind="ExternalInput")
    ident_d = nc.dram_tensor("ident", [128, 128], F32R, kind="ExternalInput")
    eup_d = nc.dram_tensor("eup", [NSLAB, 128, 128], F32R, kind="ExternalInput")
    m07_d = nc.dram_tensor("m07", [127, 1], F32, kind="ExternalInput")
    m7_d = nc.dram_tensor("m7", [128, 1], F32, kind="ExternalInput")

    out_d = nc.dram_tensor("out", [R, NSLAB, 128, S2], F32, kind="ExternalOutput")

    with tile.TileContext(nc) as tc:
        with (
            tc.tile_pool(name="consts", bufs=1) as cpool,
            tc.tile_pool(name="fields", bufs=1) as fpool,
            tc.tile_pool(name="rows", bufs=2) as rpool,
            tc.tile_pool(name="work", bufs=3) as wpool,
            tc.tile_pool(name="psA", bufs=2, space="PSUM") as psA,
            tc.tile_pool(name="psF", bufs=2, space="PSUM") as psF,
        ):
            up07_t = cpool.tile([127, 128], F32R, name="up07_t")
            nc.sync.dma_start(up07_t[:], up07_d[:, :])
            ident_t = cpool.tile([128, 128], F32R, name="ident_t")
            nc.sync.dma_start(ident_t[:], ident_d[:, :])
            eup_t = cpool.tile([128, NSLAB, 128], F32R, name="eup_t")
            nc.sync.dma_start(eup_t[:], eup_d[:, :, :].rearrange("j k c -> k j c"))
            m07_t = cpool.tile([127, 1], F32, name="m07_t")
            nc.sync.dma_start(m07_t[:], m07_d[:, :])
            m7_t = cpool.tile([128, 1], F32, name="m7_t")
            nc.sync.dma_start(m7_t[:], m7_d[:, :])

            sel07_t = cpool.tile([127, 128], F32R, name="sel07_t")
            nc.vector.tensor_scalar_mul(out=sel07_t[:],
                                        in0=up07_t[:].bitcast(F32),
                                        scalar1=m07_t[:])
            sel7_t = cpool.tile([128, 128], F32R, name="sel7_t")
            nc.vector.tensor_scalar_mul(out=sel7_t[:],
                                        in0=ident_t[:].bitcast(F32),
                                        scalar1=m7_t[:])

            coll = {}
            for nm, dd in [("A", af_d), ("B", bf_d), ("C", cf_d)]:
                f0 = fpool.tile([127, NF], F32R, name=f"{nm}f0")
                nc.sync.dma_start(f0[:], dd[0][:, :])
                f1 = fpool.tile([128, NF], F32R, name=f"{nm}f1")
                nc.sync.dma_start(f1[:], dd[1][:, :])
                cc = fpool.tile([128, NF], F32R, name=f"{nm}coll")
                for ch in range(5):
                    lo = ch * 512
                    hi = min(NF, lo + 512)
                    psc = psF.tile([128, 512], F32, name="psc")
                    nc.tensor.matmul(psc[:, :hi - lo], sel07_t[:], f0[:, lo:hi],
                                     start=True, stop=False, skip_group_check=True)
                    nc.tensor.matmul(psc[:, :hi - lo], sel7_t[:], f1[:, lo:hi],
                                     start=False, stop=True, skip_group_check=True)
                    nc.vector.tensor_copy(cc[:, lo:hi], psc[:, :hi - lo])
                coll[nm] = cc

            for r in range(R):
                xa_t = rpool.tile([128, NSLAB, S2], F32R, name="xa_t")
                nc.sync.dma_start(xa_t[:], xa_d[r].rearrange("j c s -> c j s"))
                xb_t = rpool.tile([128, NSLAB, S2], F32R, name="xb_t")
                nc.sync.dma_start(xb_t[:], xb_d[r].rearrange("j c s -> c j s"))
                out_t = rpool.tile([128, NSLAB, S2], F32, name="out_t")

                for j in range(NSLAB):
                    psa = psA.tile([128, S2], F32, name="psa")
                    nc.tensor.matmul(psa[:], eup_t[:, j, :],
                                     coll["A"][:, r * S2:(r + 1) * S2],
                                     start=True, stop=True)
                    psb = psA.tile([128, S2], F32, name="psb")
                    nc.tensor.matmul(psb[:], eup_t[:, j, :],
                                     coll["B"][:, r * S2:(r + 1) * S2],
                                     start=True, stop=True)
                    psc2 = psA.tile([128, S2], F32, name="psc2")
                    nc.tensor.matmul(psc2[:], eup_t[:, j, :],
                                     coll["C"][:, r * S2:(r + 1) * S2],
                                     start=True, stop=True)
                    t1 = wpool.tile([128, S2], F32, name="t1")
                    nc.vector.tensor_tensor(out=t1[:],
                                            in0=xa_t[:, j, :].bitcast(F32),
                                            in1=psa[:], op=OP.mult)
                    t2 = wpool.tile([128, S2], F32, name="t2")
                    nc.vector.tensor_tensor(out=t2[:],
                                            in0=xb_t[:, j, :].bitcast(F32),
                                            in1=psb[:], op=OP.mult)
                    t12 = wpool.tile([128, S2], F32, name="t12")
                    nc.gpsimd.tensor_tensor(out=t12[:], in0=t1[:], in1=t2[:],
                                            op=OP.add)
                    nc.vector.tensor_tensor(out=out_t[:, j, :], in0=t12[:],
                                            in1=psc2[:], op=OP.add)
                nc.sync.dma_start(out_d[r].rearrange("j c s -> c j s"), out_t[:])

    nc.finalize()
    return nc


def _host_inputs(x, perm):
    x = np.ascontiguousarray(np.asarray(x), dtype=np.float32)
    perm = np.asarray(perm).astype(np.int64)
    xr = np.zeros((B, NSLAB, 128, S2), dtype=np.float32)
    xr[:, :, :, :S] = _round_fp32r(x.reshape(B, C, S).copy()).reshape(
        B, NSLAB, 128, S)
    rows_per_core = [np.arange(R * k, R * (k + 1)) for k in range(NCORES)]
    xa_list = [np.ascontiguousarray(xr[rows]) for rows in rows_per_core]
    xb_list = [np.ascontiguousarray(xr[perm[rows]]) for rows in rows_per_core]
    return xa_list, xb_list, rows_per_core


def _nv_arrays(nvec):
    n = nvec
    return {
        "invsq": (1.0 / np.sqrt(n * (n - 1))).astype(np.float32).reshape(127, 1),
        "invnm1": (1.0 / (n - 1)).astype(np.float32).reshape(127, 1),
        "lnnm1": np.log(n - 1).astype(np.float32).reshape(127, 1),
        "nega": (-1.0 / n).astype(np.float32).reshape(127, 1),
        "c6n": (1.0 / (6.0 * n)).astype(np.float32).reshape(127, 1),
    }


def run_neffs(x, perm, trace=False):
    """Run both NEFFs; returns (out, info dict with exec times)."""
    from concourse.bass_utils import run_bass_kernel_spmd

    xa_list, xb_list, rows_per_core = _host_inputs(x, perm)
    ind7, eup, ind127, up07, nvec = _consts()
    nv = _nv_arrays(nvec)
    ident = np.eye(128, dtype=np.float32)

    if "n1" not in _cache:
        _cache["n1"] = _build_neff1()
    if "n2" not in _cache:
        _cache["n2"] = _build_neff2()
    n1, n2 = _cache["n1"], _cache["n2"]

    in1 = []
    for k in range(NCORES):
        m = dict(xa=xa_list[k], xb=xb_list[k], ind7=ind7, ind127=ind127,
                 ident=ident)
        for key, v in nv.items():
            m[f"nv_{key}"] = v
        in1.append(m)
    res1 = run_bass_kernel_spmd(n1, in1, core_ids=list(range(NCORES)),
                                trace=trace)

    # ---------------- host score assembly ----------------
    N = B * C * S
    q1t = sum(r["qa"].astype(np.float64).sum() for r in res1.results)
    usq = sum(r["cr"].astype(np.float64).sum() for r in res1.results)
    sxt = sum(float(r["ssum"][0, 0]) for r in res1.results)
    pt = (usq - 2.0 * q1t) / 2.0  # sum u^2 = Qa + Qb + 2P; Qa+Qb tot = 2*q1t
    q2t = q1t

    lvl_of_row = np.zeros(127, dtype=np.int64)
    for lvl in range(7):
        off = 2 ** lvl - 1
        lvl_of_row[off:off + 2 ** lvl] = lvl
    dpart = np.zeros(8, dtype=np.float64)
    for r in res1.results:
        d0 = r["dpart0"].astype(np.float64)[:, 0]
        np.add.at(dpart, lvl_of_row, d0)
        dpart[7] += r["dpart1"].astype(np.float64).sum()
    for lvl in range(8):
        gcl = 1024 >> lvl
        dpart[lvl] -= 2.0 * (gcl - 1) * (2 ** lvl) * S2 * B

    base_ss = (q1t + q2t) / 4.0 + pt / 2.0
    ss = base_ss - (EPS / 4.0) * dpart
    mean_mix = sxt / N
    scores = (ss - N * mean_mix ** 2) / (N - 1)
    order = np.argsort(-scores, kind="stable")
    sel = set(int(v) for v in order[:3])

    m07 = np.array([[1.0 if int(lvl_of_row[g]) in sel else 0.0]
                    for g in range(127)], dtype=np.float32)
    m7 = np.full((128, 1), 1.0 if 7 in sel else 0.0, dtype=np.float32)

    in2 = []
    for k in range(NCORES):
        m = dict(xa=xa_list[k], xb=xb_list[k], up07=up07, ident=ident, eup=eup,
                 m07=m07, m7=m7)
        for g in range(2):
            m[f"af{g}"] = res1.results[k][f"af{g}"]
            m[f"bf{g}"] = res1.results[k][f"bf{g}"]
            m[f"cf{g}"] = res1.results[k][f"cf{g}"]
        in2.append(m)
    res2 = run_bass_kernel_spmd(n2, in2, core_ids=list(range(NCORES)),
                                trace=trace)

    out = np.empty((B, C, H, W), dtype=np.float32)
    for k, rows in enumerate(rows_per_core):
        o = res2.results[k]["out"][:, :, :, :S]
        out[rows] = o.reshape(R, C, H, W)
    info = dict(scores=scores, sel=sorted(sel),
                t1=res1.exec_time_ns, t2=res2.exec_time_ns)
    return out, info


def kernel(x, perm):
    out, _ = run_neffs(x, perm, trace=False)
    return out


if __name__ == "__main__":
    rng = np.random.default_rng(0)
    x = rng.standard_normal((B, C, H, W), dtype=np.float32)
    perm = rng.permutation(B).astype(np.int64)
    o = kernel(x, perm)
    print("kernel ran, out shape", o.shape)


# revision 5
# speedup vs baseline: 1.0097x; 1.0097x over previous
"""Trainium2 Bass kernel for nn_CLIP_Inplanted_groupPNmixAfterConv_groupMaxNensembleOut.

Math (derived from the reference):
  For level l (g = 2**l groups, gc = 1024/g channels each),
  mix_l = a*x + b*xp + c per (b, group, s) with
    a = 0.5*s2/s1, b = 0.5*s1/s2, c = 0.5*(m1+m2) - a*m1 - b*m2.
  Identities: a*b = 1/4; sum(mix_l) is level-independent; and
    sumsq(mix_l) = (Q1t+Q2t)/4 + Pt/2 - (EPS/4)*D_l,
    D_l = sum (gc-1)*(rho + 1/rho - 2),  rho = (V2+EPS)/(V1+EPS).
  So topk-by-variance == bottomk-by-D, robustly computable in fp32, and
  out = A*x + B*xp + C with A,B,C the mean over selected levels of (a,b,c)
  broadcast to the finest 8-channel grid.

Device plan (8 cores, batch-sharded rows; perm partners gathered on host):
  slab layout [part = channel-of-slab (128), free = spatial (289)].
  NEFF1 (stats): PE fp32r indicator matmuls -> per-level S,Q; ACT exp/log
    pipeline -> a',b',c' fields (HBM spill) + per-level D partials + global
    partials.  Host: fp64 scores, stable top-3, masks.
  NEFF2 (apply): masked select matmuls collapse fields, per-slab indicator
    matmuls upsample, out = A*xa + B*xb + C on DVE/gpsimd.
Inputs are host-rounded to the fp32r (tf32) grid; PE fp32r is then exact.
"""

import numpy as np

B, C, H, W = 64, 1024, 17, 17
S = H * W            # 289
S2 = S + 1           # padded even spatial extent for fp32r matmuls
R = 8                # batch rows per core
NCORES = 8
NSLAB = 8
EPS = 1e-5
NF = R * S2          # 2320 free elems in row-batched level tiles
_cache = {}


def _round_fp32r(a, bits=13):
    ai = a.view(np.uint32).astype(np.uint64)
    half = np.uint64(1 << (bits - 1))
    mask = np.uint64(~((1 << bits) - 1) & 0xFFFFFFFF)
    return ((ai + half) & mask).astype(np.uint32).view(np.float32)


def _consts():
    ind7 = np.zeros((NSLAB, 128, 128), dtype=np.float32)
    for j in range(NSLAB):
        for c in range(128):
            ind7[j, c, 16 * j + c // 8] = 1.0
    eup = np.ascontiguousarray(ind7.transpose(0, 2, 1))
    ind127 = np.zeros((128, 127), dtype=np.float32)
    nvec = np.zeros(127, dtype=np.float64)
    for lvl in range(7):
        off = 2 ** lvl - 1
        glen = 128 >> lvl
        for i in range(2 ** lvl):
            ind127[i * glen:(i + 1) * glen, off + i] = 1.0
            nvec[off + i] = 1024 >> lvl
    up07 = np.ascontiguousarray(ind127.T)
    return ind7, eup, ind127, up07, nvec


def _build_neff1():
    import concourse.bacc as bacc
    import concourse.mybir as mybir
    import concourse.tile as tile

    F32 = mybir.dt.float32
    F32R = mybir.dt.float32r
    OP = mybir.AluOpType
    ACTF = mybir.ActivationFunctionType
    AX = mybir.AxisListType

    nc = bacc.Bacc("TRN2", target_bir_lowering=False, num_devices=NCORES)

    xa_d = nc.dram_tensor("xa", [R, 128, NSLAB, S2], F32R, kind="ExternalInput")
    xb_d = nc.dram_tensor("xb", [R, 128, NSLAB, S2], F32R, kind="ExternalInput")
    ind7_d = nc.dram_tensor("ind7", [NSLAB, 128, 128], F32R, kind="ExternalInput")
    ind127_d = nc.dram_tensor("ind127", [128, 127], F32R, kind="ExternalInput")
    ident_d = nc.dram_tensor("ident", [128, 128], F32R, kind="ExternalInput")
    nv_names = ["invsq", "invnm1", "lnnm1", "nega", "c6n"]
    nv_d = {k: nc.dram_tensor(f"nv_{k}", [127, 1], F32, kind="ExternalInput")
            for k in nv_names}

    dpart0_d = nc.dram_tensor("dpart0", [127, 1], F32, kind="ExternalOutput")
    dpart1_d = nc.dram_tensor("dpart1", [128, 1], F32, kind="ExternalOutput")
    qa_d = nc.dram_tensor("qa", [128, R], F32, kind="ExternalOutput")
    cr_d = nc.dram_tensor("cr", [128, R], F32, kind="ExternalOutput")
    ssum_d = nc.dram_tensor("ssum", [127, 1], F32, kind="ExternalOutput")
    fshape = [[127, NF], [128, NF]]
    af_d = [nc.dram_tensor(f"af{g}", fshape[g], F32, kind="ExternalOutput")
            for g in range(2)]
    bf_d = [nc.dram_tensor(f"bf{g}", fshape[g], F32, kind="ExternalOutput")
            for g in range(2)]
    cf_d = [nc.dram_tensor(f"cf{g}", fshape[g], F32, kind="ExternalOutput")
            for g in range(2)]

    with tile.TileContext(nc) as tc:
        with (
            tc.tile_pool(name="consts", bufs=1) as cpool,
            tc.tile_pool(name="rows", bufs=2) as rpool,
            tc.tile_pool(name="scr", bufs=1) as scrpool,
            tc.tile_pool(name="stats", bufs=1) as spool,
            tc.tile_pool(name="pipe", bufs=1) as ppool,
            tc.tile_pool(name="ps_l", bufs=1, space="PSUM") as ps1,
            tc.tile_pool(name="ps_u", bufs=1, space="PSUM") as ps2,
        ):
            ind7_t = cpool.tile([128, NSLAB, 128], F32R, name="ind7_t")
            nc.sync.dma_start(ind7_t[:], ind7_d[:, :, :].rearrange("j c k -> c j k"))
            ind127_t = cpool.tile([128, 127], F32R, name="ind127_t")
            nc.sync.dma_start(ind127_t[:], ind127_d[:, :])
            ident_t = cpool.tile([128, 128], F32R, name="ident_t")
            nc.sync.dma_start(ident_t[:], ident_d[:, :])
            nv_t = {}
            for k in nv_names:
                nv_t[k] = cpool.tile([127, 1], F32, name=f"nv_{k}_t")
                nc.sync.dma_start(nv_t[k][:], nv_d[k][:, :])
            eps_t = cpool.tile([128, 1], F32, name="eps_t")
            nc.vector.memset(eps_t[:], EPS)
            ln6_t = cpool.tile([128, 1], F32, name="ln6_t")
            nc.vector.memset(ln6_t[:], float(-np.log(6.0)))
            ln7_t = cpool.tile([128, 1], F32, name="ln7_t")
            nc.vector.memset(ln7_t[:], float(np.log(7.0)))

            LS = {}
            for st in ["s1", "q1", "s2", "q2"]:
                LS[(st, 0)] = spool.tile([127, NF], F32, name=f"L0_{st}")
                LS[(st, 1)] = spool.tile([128, NF], F32, name=f"L1_{st}")
            cr_sb = spool.tile([128, R], F32, name="cr_sb")
            qa_sb = spool.tile([128, R], F32, name="qa_sb")

            # ---------------- per-row stats ----------------
            for r in range(R):
                xa_t = rpool.tile([128, NSLAB, S2], F32R, name="xa_t")
                nc.sync.dma_start(xa_t[:], xa_d[r])
                xb_t = rpool.tile([128, NSLAB, S2], F32R, name="xb_t")
                nc.sync.dma_start(xb_t[:], xb_d[r])

                # cross term: u = xa + xb accumulated in PSUM via identity MMs
                psu = ps2.tile([128, NF], F32, name="psu")
                xaf = xa_t[:].rearrange("c j s -> c (j s)")
                xbf = xb_t[:].rearrange("c j s -> c (j s)")
                for ch in range(5):
                    lo = ch * 512
                    hi = min(NF, lo + 512)
                    nc.tensor.matmul(psu[:, lo:hi], ident_t[:], xaf[:, lo:hi],
                                     start=True, stop=False)
                    nc.tensor.matmul(psu[:, lo:hi], ident_t[:], xbf[:, lo:hi],
                                     start=False, stop=True)
                usq_t = scrpool.tile([128, NF], F32, name="usq_t")
                nc.scalar.activation(usq_t[:], psu[:], ACTF.Square,
                                     accum_out=cr_sb[:, r:r + 1])

                def stats_for(data_t, s_key, q_key, acc):
                    sq_t = rpool.tile([128, NSLAB, S2], F32R, name="sq_t")
                    nc.scalar.activation(sq_t[:], data_t[:].bitcast(F32),
                                         ACTF.Square, accum_out=acc)
                    for st, dat in [(s_key, data_t), (q_key, sq_t)]:
                        ps7 = ps1.tile([128, S2], F32, name="ps7")
                        for j in range(NSLAB):
                            nc.tensor.matmul(ps7[:], ind7_t[:, j, :],
                                             dat[:, j, :],
                                             start=(j == 0), stop=(j == NSLAB - 1))
                        f7dst = LS[(st, 1)][:, r * S2:(r + 1) * S2]
                        nc.vector.tensor_copy(f7dst.bitcast(F32R), ps7[:])
                        ps06 = ps1.tile([127, S2], F32, name="ps06")
                        nc.tensor.matmul(ps06[:], ind127_t[:],
                                         f7dst.bitcast(F32R),
                                         start=True, stop=True)
                        nc.scalar.copy(LS[(st, 0)][:, r * S2:(r + 1) * S2], ps06[:])

                stats_for(xa_t, "s1", "q1", qa_sb[:, r:r + 1])
                stats_for(xb_t, "s2", "q2", None)

            nc.sync.dma_start(cr_d[:, :], cr_sb[:])
            nc.sync.dma_start(qa_d[:, :], qa_sb[:])

            # ---------------- level pipeline ----------------
            for g in range(2):
                P = 127 if g == 0 else 128
                s1 = LS[("s1", g)]; q1 = LS[("q1", g)]
                s2 = LS[("s2", g)]; q2 = LS[("q2", g)]
                if g == 0:
                    invsq = nv_t["invsq"][:]
                    invnm1 = nv_t["invnm1"][:]
                    lnnm1 = nv_t["lnnm1"][:]
                    nega = nv_t["nega"][:]
                    c6n = nv_t["c6n"][:]
                else:
                    invsq = float(1.0 / np.sqrt(56.0))
                    invnm1 = float(1.0 / 7.0)
                    lnnm1 = ln7_t[:]
                    nega = float(-1.0 / 8.0)
                    c6n = float(1.0 / 48.0)

                msq = ppool.tile([128, NF], F32, name="msq")
                vA = ppool.tile([128, NF], F32, name="vA")
                vB = ppool.tile([128, NF], F32, name="vB")
                d_t = ppool.tile([128, NF], F32, name="d_t")
                ap_t = ppool.tile([128, NF], F32, name="ap_t")
                bp_t = ppool.tile([128, NF], F32, name="bp_t")

                # V1 -> ln(V1+eps) in vA
                nc.scalar.activation(msq[:P], s1[:], ACTF.Square, scale=invsq)
                nc.vector.scalar_tensor_tensor(
                    out=vA[:P], in0=q1[:], scalar=invnm1, op0=OP.mult,
                    in1=msq[:P], op1=OP.subtract)
                nc.vector.tensor_scalar_max(out=vA[:P], in0=vA[:P], scalar1=0.0)
                nc.scalar.activation(vA[:P], vA[:P], ACTF.Ln, bias=eps_t[:P])
                # V2 -> ln(V2+eps) in vB
                nc.scalar.activation(msq[:P], s2[:], ACTF.Square, scale=invsq)
                nc.vector.scalar_tensor_tensor(
                    out=vB[:P], in0=q2[:], scalar=invnm1, op0=OP.mult,
                    in1=msq[:P], op1=OP.subtract)
                nc.vector.tensor_scalar_max(out=vB[:P], in0=vB[:P], scalar1=0.0)
                nc.scalar.activation(vB[:P], vB[:P], ACTF.Ln, bias=eps_t[:P])

                nc.vector.tensor_tensor(out=d_t[:P], in0=vB[:P], in1=vA[:P],
                                        op=OP.subtract)

                # D partials: (n-1)(e^d + e^-d), -2(n-1) constant on host
                nc.scalar.activation(vA[:P], d_t[:P], ACTF.Exp, scale=1.0,
                                     bias=lnnm1)
                nc.scalar.activation(vB[:P], d_t[:P], ACTF.Exp, scale=-1.0,
                                     bias=lnnm1)
                zacc = ppool.tile([128, 1], F32, name="zacc")
                nc.vector.scalar_tensor_tensor(
                    out=msq[:P], in0=vA[:P], scalar=1.0, op0=OP.bypass,
                    in1=vB[:P], op1=OP.add, accum_out=zacc[:P])
                nc.sync.dma_start((dpart0_d if g == 0 else dpart1_d)[:, :],
                                  zacc[:P])

                # a', b'
                nc.scalar.activation(ap_t[:P], d_t[:P], ACTF.Exp, scale=0.5,
                                     bias=ln6_t[:P])
                nc.scalar.activation(bp_t[:P], d_t[:P], ACTF.Exp, scale=-0.5,
                                     bias=ln6_t[:P])
                nc.sync.dma_start(af_d[g][:, :], ap_t[:P])
                nc.sync.dma_start(bf_d[g][:, :], bp_t[:P])

                # c' = (1/(6n) - a'/n)*S1 + (1/(6n) - b'/n)*S2
                nc.vector.tensor_scalar(out=vA[:P], in0=ap_t[:P],
                                        scalar1=nega, scalar2=c6n,
                                        op0=OP.mult, op1=OP.add)
                nc.vector.tensor_scalar(out=vB[:P], in0=bp_t[:P],
                                        scalar1=nega, scalar2=c6n,
                                        op0=OP.mult, op1=OP.add)
                nc.gpsimd.tensor_tensor(out=msq[:P], in0=vA[:P], in1=s1[:],
                                        op=OP.mult)
                nc.gpsimd.tensor_tensor(out=d_t[:P], in0=vB[:P], in1=s2[:],
                                        op=OP.mult)
                nc.gpsimd.tensor_tensor(out=vA[:P], in0=msq[:P], in1=d_t[:P],
                                        op=OP.add)
                nc.sync.dma_start(cf_d[g][:, :], vA[:P])

                if g == 0:
                    ss_t = ppool.tile([127, 1], F32, name="ss_t")
                    nc.vector.reduce_sum(ss_t[:], s1[:], axis=AX.X)
                    nc.sync.dma_start(ssum_d[:, :], ss_t[:])

    nc.finalize()
    return nc


def _build_neff2():
    import concourse.bacc as bacc
    import concourse.mybir as mybir
    import concourse.tile as tile

    F32 = mybir.dt.float32
    F32R = mybir.dt.float32r
    OP = mybir.AluOpType

    nc = bacc.Bacc("TRN2", target_bir_lowering=False, num_devices=NCORES)

    xa_d = nc.dram_tensor("xa", [R, 128, NSLAB, S2], F32R, kind="ExternalInput")
    xb_d = nc.dram_tensor("xb", [R, 128, NSLAB, S2], F32R, kind="ExternalInput")
    fshape = [[127, NF], [128, NF]]
    af_d = [nc.dram_tensor(f"af{g}", fshape[g], F32R, kind="ExternalInput")
            for g in range(2)]
    bf_d = [nc.dram_tensor(f"bf{g}", fshape[g], F32R, kind="ExternalInput")
            for g in range(2)]
    cf_d = [nc.dram_tensor(f"cf{g}", fshape[g], F32R, kind="ExternalInput")
            for g in range(2)]
    up07_d = nc.dram_tensor("up07", [127, 128], F32R, kind="ExternalInput")
    ident_d = nc.dram_tensor("ident", [128, 128], F32R, kind="ExternalInput")
    eup_d = nc.dram_tensor("eup", [NSLAB, 128, 128], F32R, kind="ExternalInput")
    m07_d = nc.dram_tensor("m07", [127, 1], F32, kind="ExternalInput")
    m7_d = nc.dram_tensor("m7", [128, 1], F32, kind="ExternalInput")

    out_d = nc.dram_tensor("out", [R, 128, NSLAB, S2], F32, kind="ExternalOutput")

    with tile.TileContext(nc) as tc:
        with (
            tc.tile_pool(name="consts", bufs=1) as cpool,
            tc.tile_pool(name="fields", bufs=1) as fpool,
            tc.tile_pool(name="rows", bufs=2) as rpool,
            tc.tile_pool(name="work", bufs=3) as wpool,
            tc.tile_pool(name="psA", bufs=2, space="PSUM") as psA,
            tc.tile_pool(name="psF", bufs=2, space="PSUM") as psF,
        ):
            up07_t = cpool.tile([127, 128], F32R, name="up07_t")
            nc.sync.dma_start(up07_t[:], up07_d[:, :])
            ident_t = cpool.tile([128, 128], F32R, name="ident_t")
            nc.sync.dma_start(ident_t[:], ident_d[:, :])
            eup_t = cpool.tile([128, NSLAB, 128], F32R, name="eup_t")
            nc.sync.dma_start(eup_t[:], eup_d[:, :, :].rearrange("j k c -> k j c"))
            m07_t = cpool.tile([127, 1], F32, name="m07_t")
            nc.sync.dma_start(m07_t[:], m07_d[:, :])
            m7_t = cpool.tile([128, 1], F32, name="m7_t")
            nc.sync.dma_start(m7_t[:], m7_d[:, :])

            sel07_t = cpool.tile([127, 128], F32R, name="sel07_t")
            nc.vector.tensor_scalar_mul(out=sel07_t[:],
                                        in0=up07_t[:].bitcast(F32),
                                        scalar1=m07_t[:])
            sel7_t = cpool.tile([128, 128], F32R, name="sel7_t")
            nc.vector.tensor_scalar_mul(out=sel7_t[:],
                                        in0=ident_t[:].bitcast(F32),
                                        scalar1=m7_t[:])

            coll = {}
            for nm, dd in [("A", af_d), ("B", bf_d), ("C", cf_d)]:
                f0 = fpool.tile([127, NF], F32R, name=f"{nm}f0")
                nc.sync.dma_start(f0[:], dd[0][:, :])
                f1 = fpool.tile([128, NF], F32R, name=f"{nm}f1")
                nc.sync.dma_start(f1[:], dd[1][:, :])
                cc = fpool.tile([128, NF], F32R, name=f"{nm}coll")
                for ch in range(5):
                    lo = ch * 512
                    hi = min(NF, lo + 512)
                    psc = psF.tile([128, 512], F32, name="psc")
                    nc.tensor.matmul(psc[:, :hi - lo], sel07_t[:], f0[:, lo:hi],
                                     start=True, stop=False, skip_group_check=True)
                    nc.tensor.matmul(psc[:, :hi - lo], sel7_t[:], f1[:, lo:hi],
                                     start=False, stop=True, skip_group_check=True)
                    nc.vector.tensor_copy(cc[:, lo:hi], psc[:, :hi - lo])
                coll[nm] = cc

            for r in range(R):
                xa_t = rpool.tile([128, NSLAB, S2], F32R, name="xa_t")
                nc.sync.dma_start(xa_t[:], xa_d[r])
                xb_t = rpool.tile([128, NSLAB, S2], F32R, name="xb_t")
                nc.sync.dma_start(xb_t[:], xb_d[r])
                out_t = rpool.tile([128, NSLAB, S2], F32, name="out_t")

                for j in range(NSLAB):
                    psa = psA.tile([128, S2], F32, name="psa")
                    nc.tensor.matmul(psa[:], eup_t[:, j, :],
                                     coll["A"][:, r * S2:(r + 1) * S2],
                                     start=True, stop=True)
                    psb = psA.tile([128, S2], F32, name="psb")
                    nc.tensor.matmul(psb[:], eup_t[:, j, :],
                                     coll["B"][:, r * S2:(r + 1) * S2],
                                     start=True, stop=True)
                    psc2 = psA.tile([128, S2], F32, name="psc2")
                    nc.tensor.matmul(psc2[:], eup_t[:, j, :],
                                     coll["C"][:, r * S2:(r + 1) * S2],
                                     start=True, stop=True)
                    t1 = wpool.tile([128, S2], F32, name="t1")
                    nc.vector.tensor_tensor(out=t1[:],
                                            in0=xa_t[:, j, :].bitcast(F32),
                                            in1=psa[:], op=OP.mult)
                    t2 = wpool.tile([128, S2], F32, name="t2")
                    nc.vector.tensor_tensor(out=t2[:],
                                            in0=xb_t[:, j, :].bitcast(F32),
                                            in1=psb[:], op=OP.mult)
                    t12 = wpool.tile([128, S2], F32, name="t12")
                    nc.gpsimd.tensor_tensor(out=t12[:], in0=t1[:], in1=t2[:],
                                            op=OP.add)
                    nc.vector.tensor_tensor(out=out_t[:, j, :], in0=t12[:],
                                            in1=psc2[:], op=OP.add)
                nc.sync.dma_start(out_d[r], out_t[:])

    nc.finalize()
    return nc


def _host_inputs(x, perm):
    x = np.ascontiguousarray(np.asarray(x), dtype=np.float32)
    perm = np.asarray(perm).astype(np.int64)
    xr = np.zeros((B, 128, NSLAB, S2), dtype=np.float32)
    # [B, NSLAB, 128, S] -> [B, 128(c-of-slab), NSLAB, S]
    xr[:, :, :, :S] = _round_fp32r(x.reshape(B, C, S).copy()).reshape(
        B, NSLAB, 128, S).transpose(0, 2, 1, 3)
    rows_per_core = [np.arange(R * k, R * (k + 1)) for k in range(NCORES)]
    xa_list = [np.ascontiguousarray(xr[rows]) for rows in rows_per_core]
    xb_list = [np.ascontiguousarray(xr[perm[rows]]) for rows in rows_per_core]
    return xa_list, xb_list, rows_per_core


def _nv_arrays(nvec):
    n = nvec
    return {
        "invsq": (1.0 / np.sqrt(n * (n - 1))).astype(np.float32).reshape(127, 1),
        "invnm1": (1.0 / (n - 1)).astype(np.float32).reshape(127, 1),
        "lnnm1": np.log(n - 1).astype(np.float32).reshape(127, 1),
        "nega": (-1.0 / n).astype(np.float32).reshape(127, 1),
        "c6n": (1.0 / (6.0 * n)).astype(np.float32).reshape(127, 1),
    }


def run_neffs(x, perm, trace=False):
    """Run both NEFFs; returns (out, info dict with exec times)."""
    from concourse.bass_utils import run_bass_kernel_spmd

    xa_list, xb_list, rows_per_core = _host_inputs(x, perm)
    ind7, eup, ind127, up07, nvec = _consts()
    nv = _nv_arrays(nvec)
    ident = np.eye(128, dtype=np.float32)

    if "n1" not in _cache:
        _cache["n1"] = _build_neff1()
    if "n2" not in _cache:
        _cache["n2"] = _build_neff2()
    n1, n2 = _cache["n1"], _cache["n2"]

    in1 = []
    for k in range(NCORES):
        m = dict(xa=xa_list[k], xb=xb_list[k], ind7=ind7, ind127=ind127,
                 ident=ident)
        for key, v in nv.items():
            m[f"nv_{key}"] = v
        in1.append(m)
    res1 = run_bass_kernel_spmd(n1, in1, core_ids=list(range(NCORES)),
                                trace=trace)

    # ---------------- host score assembly ----------------
    N = B * C * S
    q1t = sum(r["qa"].astype(np.float64).sum() for r in res1.results)
    usq = sum(r["cr"].astype(np.float64).sum() for r in res1.results)
    sxt = sum(float(r["ssum"][0, 0]) for r in res1.results)
    pt = (usq - 2.0 * q1t) / 2.0  # sum u^2 = Qa + Qb + 2P; Qa+Qb tot = 2*q1t
    q2t = q1t

    lvl_of_row = np.zeros(127, dtype=np.int64)
    for lvl in range(7):
        off = 2 ** lvl - 1
        lvl_of_row[off:off + 2 ** lvl] = lvl
    dpart = np.zeros(8, dtype=np.float64)
    for r in res1.results:
        d0 = r["dpart0"].astype(np.float64)[:, 0]
        np.add.at(dpart, lvl_of_row, d0)
        dpart[7] += r["dpart1"].astype(np.float64).sum()
    for lvl in range(8):
        gcl = 1024 >> lvl
        dpart[lvl] -= 2.0 * (gcl - 1) * (2 ** lvl) * S2 * B

    base_ss = (q1t + q2t) / 4.0 + pt / 2.0
    ss = base_ss - (EPS / 4.0) * dpart
    mean_mix = sxt / N
    scores = (ss - N * mean_mix ** 2) / (N - 1)
    order = np.argsort(-scores, kind="stable")
    sel = set(int(v) for v in order[:3])

    m07 = np.array([[1.0 if int(lvl_of_row[g]) in sel else 0.0]
                    for g in range(127)], dtype=np.float32)
    m7 = np.full((128, 1), 1.0 if 7 in sel else 0.0, dtype=np.float32)

    in2 = []
    for k in range(NCORES):
        m = dict(xa=xa_list[k], xb=xb_list[k], up07=up07, ident=ident, eup=eup,
                 m07=m07, m7=m7)
        for g in range(2):
            m[f"af{g}"] = res1.results[k][f"af{g}"]
            m[f"bf{g}"] = res1.results[k][f"bf{g}"]
            m[f"cf{g}"] = res1.results[k][f"cf{g}"]
        in2.append(m)
    res2 = run_bass_kernel_spmd(n2, in2, core_ids=list(range(NCORES)),
                                trace=trace)

    out = np.empty((B, C, H, W), dtype=np.float32)
    for k, rows in enumerate(rows_per_core):
        o = res2.results[k]["out"][:, :, :, :S]  # [R, 128, NSLAB, S]
        out[rows] = o.transpose(0, 2, 1, 3).reshape(R, C, H, W)
    info = dict(scores=scores, sel=sorted(sel),
                t1=res1.exec_time_ns, t2=res2.exec_time_ns)
    return out, info


def kernel(x, perm):
    out, _ = run_neffs(x, perm, trace=False)
    return out


if __name__ == "__main__":
    rng = np.random.default_rng(0)
    x = rng.standard_normal((B, C, H, W), dtype=np.float32)
    perm = rng.permutation(B).astype(np.int64)
    o = kernel(x, perm)
    print("kernel ran, out shape", o.shape)
